# revision 1
# baseline (speedup 1.0000x reference)
"""AttentiveFP readout kernel for 8 Trainium2 NeuronCores.

Strategy: graph-contiguous sharding of the V=500k nodes across 8 cores
(seg_ids sorted => each graph's nodes contiguous; split at graph
boundaries nearest V/8 multiples). Every graph lives entirely on one
core, so all segment ops are core-local and no collectives are needed.

Per core: graphs are processed in tiles of 128 (partition dim = graph).
Each graph-tile's nodes (<= NSUB*128, host-padded) are streamed through
SBUF once. Segment sum / weighted segment sum are TensorEngine matmuls
against a one-hot node->graph membership matrix built on-device via
iota==segrel compare. The attention-weighted projection uses
  g_repr = (sum_v a_v * x_v) @ Wp.T + bp   (since sum_v a_v = 1)
so the only V-sized matmuls are the K=128 one-hot reductions.
Segment softmax skips the max-subtraction (|z| <~ 12, exp is safe in
fp32). GRU runs per 128-graph tile on-chip.
"""

import numpy as np
from contextlib import ExitStack

import concourse.bass as bass
import concourse.bacc as bacc
import concourse.mybir as mybir
from concourse import tile
from concourse.bass_utils import run_bass_kernel_spmd

F32 = mybir.dt.float32
BF16 = mybir.dt.bfloat16
NP_BF16 = mybir.dt.np(mybir.dt.bfloat16)
AOP = mybir.AluOpType
ACT = mybir.ActivationFunctionType
AX = mybir.AxisListType

NCORES = 8
F = 256
T = 2
GT = 128  # graphs per tile (partition dim)
LAST_RESULT = None


def _build_program(NT_G, NSUB, bl_vals, stage=99):
    """Build the per-core SPMD program. Returns (nc, ctx).
    stage: debug cutoff; 99 = full program."""
    ctx = ExitStack()
    nc = bacc.Bacc("TRN2")
    epsc = nc.alloc_sbuf_tensor("const-f32-eps", [128, 1], F32)
    nc.gpsimd.memset(epsc.ap(), 1e-30)
    blc = []
    for t in range(T):
        bt = nc.alloc_sbuf_tensor(f"const-f32-bl{t}", [128, 1], F32)
        nc.gpsimd.memset(bt.ap(), float(bl_vals[t]))
        blc.append(bt)
    nc.all_engine_barrier()

    nf_d = nc.dram_tensor("nf", [NT_G * NSUB * 128, F], F32, kind="ExternalInput")
    segrel_d = nc.dram_tensor("segrel", [NT_G * 128, NSUB], F32, kind="ExternalInput")
    iota_d = nc.dram_tensor("iota", [128, 128], F32, kind="ExternalInput")
    identb_d = nc.dram_tensor("identb", [128, 128], BF16, kind="ExternalInput")
    identf_d = nc.dram_tensor("identf", [128, 128], F32, kind="ExternalInput")
    ones1_d = nc.dram_tensor("ones1", [1, 128], F32, kind="ExternalInput")
    wlg_d = [nc.dram_tensor(f"wlg{t}", [128, F], BF16, kind="ExternalInput") for t in range(T)]
    wln_d = [nc.dram_tensor(f"wln{t}", [128, F], BF16, kind="ExternalInput") for t in range(T)]
    wpt_d = [nc.dram_tensor(f"wpt{t}", [F, F], BF16, kind="ExternalInput") for t in range(T)]
    wih_d = [nc.dram_tensor(f"wiht{t}", [F, 3 * F], BF16, kind="ExternalInput") for t in range(T)]
    whh_d = [nc.dram_tensor(f"whht{t}", [F, 3 * F], BF16, kind="ExternalInput") for t in range(T)]
    brz_d = [nc.dram_tensor(f"brz{t}", [128, 2 * F], F32, kind="ExternalInput") for t in range(T)]
    bin_d = [nc.dram_tensor(f"bin{t}", [128, F], F32, kind="ExternalInput") for t in range(T)]
    bhn_d = [nc.dram_tensor(f"bhn{t}", [128, F], F32, kind="ExternalInput") for t in range(T)]
    bpb_d = [nc.dram_tensor(f"bpb{t}", [128, F], F32, kind="ExternalInput") for t in range(T)]
    out_d = nc.dram_tensor("out", [NT_G * 128, F], F32, kind="ExternalOutput")

    with tile.TileContext(nc) as tc:
      with tc.sbuf_pool(name="const", bufs=1) as cpool, \
           tc.sbuf_pool(name="work", bufs=2) as wpool, \
           tc.sbuf_pool(name="small", bufs=2) as spool, \
           tc.sbuf_pool(name="scr", bufs=3) as scrpool, \
           tc.sbuf_pool(name="stage", bufs=6) as stpool, \
           tc.psum_pool(name="pacc", bufs=2) as pacc, \
           tc.psum_pool(name="prz", bufs=2) as prz, \
           tc.psum_pool(name="ptiny", bufs=4) as ptiny:

        iota_sb = cpool.tile_from(iota_d[:, :], name="iota_sb")
        identb_sb = cpool.tile_from(identb_d[:, :], name="identb_sb")
        identf_sb = cpool.tile_from(identf_d[:, :], name="identf_sb")
        ones1_sb = cpool.tile_from(ones1_d[:, :], name="ones1_sb")
        wlg_sb = [cpool.tile_from(wlg_d[t][:, :], name=f"wlg_sb{t}") for t in range(T)]
        wln_sb = [cpool.tile_from(wln_d[t][:, :], name=f"wln_sb{t}") for t in range(T)]
        brz_sb = [cpool.tile_from(brz_d[t][:, :], name=f"brz_sb{t}") for t in range(T)]
        bin_sb = [cpool.tile_from(bin_d[t][:, :], name=f"bin_sb{t}") for t in range(T)]
        bhn_sb = [cpool.tile_from(bhn_d[t][:, :], name=f"bhn_sb{t}") for t in range(T)]
        bpb_sb = [cpool.tile_from(bpb_d[t][:, :], name=f"bpb_sb{t}") for t in range(T)]
        # K-chunked weights: [128, 2, N] with chunk k = rows k*128..k*128+128
        wpt_sb, wih_sb, whh_sb = [], [], []
        for t in range(T):
            wp_t = cpool.tile([128, 2, F], BF16, name=f"wp_sb{t}")
            wi_t = cpool.tile([128, 2, 3 * F], BF16, name=f"wi_sb{t}")
            wh_t = cpool.tile([128, 2, 3 * F], BF16, name=f"wh_sb{t}")
            for k in range(2):
                nc.sync.dma_start(wp_t[:, k, :], wpt_d[t][k * 128:(k + 1) * 128, :])
                nc.sync.dma_start(wi_t[:, k, :], wih_d[t][k * 128:(k + 1) * 128, :])
                nc.sync.dma_start(wh_t[:, k, :], whh_d[t][k * 128:(k + 1) * 128, :])
            wpt_sb.append(wp_t)
            wih_sb.append(wi_t)
            whh_sb.append(wh_t)

        for j in range(NT_G):
            segrel_sb = wpool.tile([128, NSUB], F32, name=f"segrel_{j}", tag="segrel")
            nc.sync.dma_start(segrel_sb[:, :], segrel_d[j * 128:(j + 1) * 128, :])
            nf_aug = wpool.tile([128, NSUB, F + 1], BF16, name=f"nfaug_{j}", tag="nfaug")
            Mn = wpool.tile([128, NSUB, 128], BF16, name=f"Mn_{j}", tag="Mn")
            nc.gpsimd.memset(nf_aug[:, :, 0], 1.0)
            ps_g0 = pacc.tile([128, F + 1], F32, name=f"psg0_{j}", tag="acc")
            for s in range(NSUB):
                stg = stpool.tile([128, F], F32, name=f"stg_{j}_{s}", tag="stage")
                r0 = (j * NSUB + s) * 128
                nc.sync.dma_start(stg[:, :], nf_d[r0:r0 + 128, :])
                if s % 2 == 0:
                    nc.vector.tensor_copy(nf_aug[:, s, 1:F + 1], stg[:, :])
                else:
                    nc.scalar.copy(nf_aug[:, s, 1:F + 1], stg[:, :])
                nc.vector.tensor_tensor(
                    Mn[:, s, :], segrel_sb[:, s:s + 1].broadcast_to((128, 128)),
                    iota_sb[:, :], op=AOP.is_equal)
                nc.tensor.matmul(ps_g0[:, 0:F], Mn[:, s, :],
                                 nf_aug[:, s, 1:F + 1],
                                 start=(s == 0), stop=(s == NSUB - 1))
            # per-node logits' node part: w[t][:, s] = nf . wl_n[t]
            w01 = wpool.tile([128, T, NSUB], F32, name=f"w01_{j}", tag="w01")
            for t in range(T):
                scrw = scrpool.tile([128, NSUB, F], BF16, name=f"scrw_{j}_{t}", tag="scr")
                nc.vector.tensor_tensor(
                    scrw[:, :, :], nf_aug[:, :, 1:F + 1],
                    wln_sb[t][:, :].unsqueeze(1).broadcast_to((128, NSUB, F)),
                    op=AOP.mult)
                nc.vector.reduce_sum(w01[:, t, :], scrw[:, :, :], axis=AX.X)
            gf = spool.tile([128, F], F32, name=f"gf0_{j}", tag="gf", bufs=6)
            nc.scalar.copy(gf[:, :], ps_g0[:, 0:F])

            if stage <= 1:
                nc.sync.dma_start(out_d[j * 128:(j + 1) * 128, :], gf[:, :])
                continue
            for t in range(T):
                # u_g = relu(gf) . wl_g   (per graph), broadcast to nodes
                rgf = spool.tile([128, F], BF16, name=f"rgf_{j}_{t}", tag="rgf")
                nc.scalar.activation(rgf[:, :], gf[:, :], ACT.Relu)
                ucol = spool.tile([128, 1], F32, name=f"ucol_{j}_{t}", tag="ucol")
                uscr = scrpool.tile([128, F], BF16, name=f"uscr_{j}_{t}", tag="uscr")
                nc.vector.tensor_tensor(uscr[:, :], rgf[:, :],
                                        wlg_sb[t][:, :], op=AOP.mult)
                nc.vector.reduce_sum(ucol[:, :], uscr[:, :], axis=AX.X)
                if stage <= 11:
                    nc.vector.tensor_copy(gf[:, 0:1], ucol[:, :])
                    continue
                urow_ps = ptiny.tile([1, 128], F32, name=f"urps_{j}_{t}", tag="tiny")
                nc.tensor.transpose(urow_ps[:, :], ucol[:, :], identf_sb[:, :])
                urow = spool.tile([1, 128], F32, name=f"urow_{j}_{t}", tag="urow")
                nc.scalar.copy(urow[:, :], urow_ps[:, :])
                if stage <= 12:
                    nc.vector.tensor_copy(gf[0:1, :], urow[:, :])
                    continue
                ubc_ps = ptiny.tile([128, 128], F32, name=f"ubcps_{j}_{t}", tag="tiny")
                nc.tensor.matmul(ubc_ps[:, :], ones1_sb[:, :], urow[:, :],
                                 start=True, stop=True)
                ubc = spool.tile([128, 128], BF16, name=f"ubc_{j}_{t}", tag="ubc")
                nc.scalar.copy(ubc[:, :], ubc_ps[:, :])
                if stage <= 13:
                    nc.vector.tensor_copy(gf[:, 0:128], ubc[:, :])
                    continue
                scr2 = scrpool.tile([128, NSUB, 128], BF16, name=f"scr2_{j}_{t}", tag="scr")
                nc.vector.tensor_tensor(
                    scr2[:, :, :], Mn[:, :, :],
                    ubc[:, :].unsqueeze(1).broadcast_to((128, NSUB, 128)),
                    op=AOP.mult)
                ubcv = spool.tile([128, NSUB], F32, name=f"ubcv_{j}_{t}", tag="ubcv")
                nc.vector.reduce_sum(ubcv[:, :], scr2[:, :, :], axis=AX.X)
                if stage <= 14:
                    nc.vector.tensor_copy(gf[:, 0:NSUB], ubcv[:, :])
                    continue
                zt0 = spool.tile([128, NSUB], F32, name=f"zt0_{j}_{t}", tag="zt0")
                nc.vector.tensor_tensor(zt0[:, :], ubcv[:, :], w01[:, t, :],
                                        op=AOP.add)
                zt = spool.tile([128, NSUB], F32, name=f"zt_{j}_{t}", tag="zt")
                nc.vector.tensor_tensor(zt[:, :], zt0[:, :],
                                        blc[t].ap().broadcast_to((128, NSUB)),
                                        op=AOP.add)
                zs = spool.tile([128, NSUB], F32, name=f"zs_{j}_{t}", tag="zs")
                nc.scalar.mul(zs[:, :], zt[:, :], 0.01)
                zl = spool.tile([128, NSUB], F32, name=f"zl_{j}_{t}", tag="zl")
                nc.vector.tensor_tensor(zl[:, :], zt[:, :], zs[:, :], op=AOP.max)
                ebf = spool.tile([128, NSUB], BF16, name=f"ebf_{j}_{t}", tag="ebf")
                nc.scalar.activation(ebf[:, :], zl[:, :], ACT.Exp)
                if stage <= 2:
                    nc.vector.tensor_copy(gf[:, 0:NSUB], ebf[:, :])
                    continue
                # weighted per-node features [e | e*x] and segment-reduce
                scr3 = scrpool.tile([128, NSUB, F + 1], BF16, name=f"scr3_{j}_{t}", tag="scr")
                nc.vector.tensor_tensor(
                    scr3[:, :, :], nf_aug[:, :, :],
                    ebf[:, :].unsqueeze(2).broadcast_to((128, NSUB, F + 1)),
                    op=AOP.mult)
                ps_ds = pacc.tile([128, F + 1], F32, name=f"psds_{j}_{t}", tag="acc")
                for s in range(NSUB):
                    nc.tensor.matmul(ps_ds[:, :], Mn[:, s, :], scr3[:, s, :],
                                     start=(s == 0), stop=(s == NSUB - 1))
                dplus = spool.tile([128, 1], F32, name=f"dplus_{j}_{t}", tag="dplus")
                nc.vector.tensor_tensor(dplus[:, :], ps_ds[:, 0:1], epsc.ap(),
                                        op=AOP.max)
                recd = spool.tile([128, 1], F32, name=f"recd_{j}_{t}", tag="recd")
                nc.vector.reciprocal(recd[:, :], dplus[:, :])
                stl = spool.tile([128, F], BF16, name=f"stl_{j}_{t}", tag="stl")
                nc.vector.tensor_tensor(stl[:, :], ps_ds[:, 1:F + 1],
                                        recd[:, :].broadcast_to((128, F)),
                                        op=AOP.mult)
                if stage <= 3:
                    nc.vector.tensor_copy(gf[:, :], stl[:, :])
                    continue
                # g_repr = stl @ Wp.T  (via transposed stl chunks)
                stT = spool.tile([128, 2, 128], BF16, name=f"stT_{j}_{t}", tag="stT")
                for k in range(2):
                    pst = ptiny.tile([128, 128], BF16, name=f"pst_{j}_{t}_{k}", tag="tiny")
                    nc.tensor.transpose(pst[:, :], stl[:, k * 128:(k + 1) * 128],
                                        identb_sb[:, :])
                    nc.scalar.copy(stT[:, k, :], pst[:, :])
                ps_wp = ptiny.tile([128, F], F32, name=f"pswp_{j}_{t}", tag="tiny")
                for k in range(2):
                    nc.tensor.matmul(ps_wp[:, :], stT[:, k, :], wpt_sb[t][:, k, :],
                                     start=(k == 0), stop=(k == 1))
                # context = elu(g_repr + bp) = relu(x) + exp(min(x,0)) - 1
                xg = spool.tile([128, F], F32, name=f"xg_{j}_{t}", tag="xg")
                nc.vector.tensor_tensor(xg[:, :], ps_wp[:, :], bpb_sb[t][:, :], op=AOP.add)
                xn = spool.tile([128, F], F32, name=f"xn_{j}_{t}", tag="xn")
                nc.vector.tensor_tensor(xn[:, :], xg[:, :],
                                        nc.const_aps.tensor(0.0, (128, F)),
                                        op=AOP.min)
                en = spool.tile([128, F], F32, name=f"en_{j}_{t}", tag="en")
                nc.scalar.activation(en[:, :], xn[:, :], ACT.Exp)
                xp = spool.tile([128, F], F32, name=f"xp_{j}_{t}", tag="xp")
                nc.scalar.activation(xp[:, :], xg[:, :], ACT.Relu)
                s1 = spool.tile([128, F], F32, name=f"s1_{j}_{t}", tag="s1")
                nc.vector.tensor_tensor(s1[:, :], en[:, :], xp[:, :], op=AOP.add)
                ctxb = spool.tile([128, F], BF16, name=f"ctxb_{j}_{t}", tag="ctxb")
                nc.vector.tensor_tensor(ctxb[:, :], s1[:, :],
                                        nc.const_aps.tensor(1.0, (128, F)),
                                        op=AOP.subtract)
                if stage <= 4:
                    nc.vector.tensor_copy(gf[:, :], ctxb[:, :])
                    continue
                # GRU(x=ctxb, h=gf)
                gfb = spool.tile([128, F], BF16, name=f"gfb_{j}_{t}", tag="gfb")
                nc.scalar.copy(gfb[:, :], gf[:, :])
                xT = spool.tile([128, 2, 128], BF16, name=f"xT_{j}_{t}", tag="xT")
                hT = spool.tile([128, 2, 128], BF16, name=f"hT_{j}_{t}", tag="hT")
                for k in range(2):
                    p1 = ptiny.tile([128, 128], BF16, name=f"p1_{j}_{t}_{k}", tag="tiny")
                    nc.tensor.transpose(p1[:, :], ctxb[:, k * 128:(k + 1) * 128],
                                        identb_sb[:, :])
                    nc.scalar.copy(xT[:, k, :], p1[:, :])
                    p2 = ptiny.tile([128, 128], BF16, name=f"p2_{j}_{t}_{k}", tag="tiny")
                    nc.tensor.transpose(p2[:, :], gfb[:, k * 128:(k + 1) * 128],
                                        identb_sb[:, :])
                    nc.scalar.copy(hT[:, k, :], p2[:, :])
                ps_rz = prz.tile([128, 2 * F], F32, name=f"psrz_{j}_{t}", tag="rz")
                mm = 0
                for lhsT, wt in ((xT, wih_sb[t]), (hT, whh_sb[t])):
                    for k in range(2):
                        nc.tensor.matmul(ps_rz[:, :], lhsT[:, k, :],
                                         wt[:, k, 0:2 * F],
                                         start=(mm == 0), stop=(mm == 3))
                        mm += 1
                ps_in = ptiny.tile([128, F], F32, name=f"psin_{j}_{t}", tag="tiny")
                for k in range(2):
                    nc.tensor.matmul(ps_in[:, :], xT[:, k, :],
                                     wih_sb[t][:, k, 2 * F:3 * F],
                                     start=(k == 0), stop=(k == 1))
                ps_hn = ptiny.tile([128, F], F32, name=f"pshn_{j}_{t}", tag="tiny")
                for k in range(2):
                    nc.tensor.matmul(ps_hn[:, :], hT[:, k, :],
                                     whh_sb[t][:, k, 2 * F:3 * F],
                                     start=(k == 0), stop=(k == 1))
                rzs = spool.tile([128, 2 * F], F32, name=f"rzs_{j}_{t}", tag="rzs")
                nc.vector.tensor_tensor(rzs[:, :], ps_rz[:, :], brz_sb[t][:, :], op=AOP.add)
                rza = spool.tile([128, 2 * F], F32, name=f"rza_{j}_{t}", tag="rza")
                nc.scalar.activation(rza[:, :], rzs[:, :], ACT.Sigmoid)
                hns = spool.tile([128, F], F32, name=f"hns_{j}_{t}", tag="hns")
                nc.vector.tensor_tensor(hns[:, :], ps_hn[:, :], bhn_sb[t][:, :], op=AOP.add)
                tmp = spool.tile([128, F], F32, name=f"tmp_{j}_{t}", tag="tmp")
                nc.vector.tensor_tensor(tmp[:, :], rza[:, 0:F], hns[:, :], op=AOP.mult)
                t2 = spool.tile([128, F], F32, name=f"t2_{j}_{t}", tag="t2")
                nc.vector.tensor_tensor(t2[:, :], tmp[:, :], ps_in[:, :], op=AOP.add)
                t3 = spool.tile([128, F], F32, name=f"t3_{j}_{t}", tag="t3")
                nc.vector.tensor_tensor(t3[:, :], t2[:, :], bin_sb[t][:, :], op=AOP.add)
                nn = spool.tile([128, F], F32, name=f"nn_{j}_{t}", tag="nn")
                nc.scalar.activation(nn[:, :], t3[:, :], ACT.Tanh)
                hm = spool.tile([128, F], F32, name=f"hm_{j}_{t}", tag="hm")
                nc.vector.tensor_tensor(hm[:, :], gf[:, :], nn[:, :], op=AOP.subtract)
                hz = spool.tile([128, F], F32, name=f"hz_{j}_{t}", tag="hz")
                nc.vector.tensor_tensor(hz[:, :], hm[:, :], rza[:, F:2 * F], op=AOP.mult)
                gf_new = spool.tile([128, F], F32, name=f"gfn_{j}_{t}", tag="gf", bufs=6)
                nc.vector.tensor_tensor(gf_new[:, :], hz[:, :], nn[:, :], op=AOP.add)
                gf = gf_new
            nc.sync.dma_start(out_d[j * 128:(j + 1) * 128, :], gf[:, :])
    nc.finalize()
    return nc, ctx


def _prep_core(node_feats, seg, g_lo, g_hi, n_lo, n_hi, NT_G, NSUB):
    """Build padded nf / segrel arrays for one core."""
    nf_pad = np.zeros((NT_G * NSUB * 128, F), np.float32)
    segrel = np.full((NT_G * 128, NSUB), -1.0, np.float32)
    for j in range(NT_G):
        gt = g_lo + j * 128
        if gt >= g_hi:
            continue
        ge = min(gt + 128, g_hi)
        a = int(np.searchsorted(seg, gt, 'left'))
        b = int(np.searchsorted(seg, ge, 'left'))
        cnt = b - a
        assert cnt <= NSUB * 128
        nf_pad[j * NSUB * 128: j * NSUB * 128 + cnt] = node_feats[a:b]
        rel = np.full(NSUB * 128, -1.0, np.float32)
        rel[:cnt] = (seg[a:b] - gt).astype(np.float32)
        # segrel[j*128 + p, s] = rel of node s*128+p
        segrel[j * 128:(j + 1) * 128, :] = rel.reshape(NSUB, 128).T
    return nf_pad, segrel


def kernel(node_feats, seg_ids, Wl, bl, Wp, bp, Wih, Whh, bih, bhh):
    node_feats = np.asarray(node_feats, np.float32)
    seg = np.asarray(seg_ids).astype(np.int64)
    Wl = np.asarray(Wl, np.float32)
    bl = np.asarray(bl, np.float32)
    Wp = np.asarray(Wp, np.float32)
    bp = np.asarray(bp, np.float32)
    Wih = np.asarray(Wih, np.float32)
    Whh = np.asarray(Whh, np.float32)
    bih = np.asarray(bih, np.float32)
    bhh = np.asarray(bhh, np.float32)
    V = node_feats.shape[0]
    G = 25000

    # graph-contiguous shard boundaries
    bounds_g = [0]
    for c in range(1, NCORES):
        bounds_g.append(int(seg[c * V // NCORES]))
    bounds_g.append(G)
    bounds_n = [int(np.searchsorted(seg, g, 'left')) for g in bounds_g]

    NT_G = max((bounds_g[c + 1] - bounds_g[c] + 127) // 128 for c in range(NCORES))
    maxnodes = 1
    for c in range(NCORES):
        for gt in range(bounds_g[c], bounds_g[c + 1], 128):
            ge = min(gt + 128, bounds_g[c + 1])
            a = np.searchsorted(seg, gt, 'left')
            b = np.searchsorted(seg, ge, 'left')
            maxnodes = max(maxnodes, int(b - a))
    NSUB = (maxnodes + 127) // 128

    nc, ctx = _build_program(NT_G, NSUB, [float(bl[t, 0]) for t in range(T)])

    # shared (replicated) weight arrays
    shared = {
        "iota": np.broadcast_to(np.arange(128, dtype=np.float32), (128, 128)).copy(),
        "identb": np.eye(128, dtype=np.float32).astype(NP_BF16),
        "identf": np.eye(128, dtype=np.float32),
        "ones1": np.ones((1, 128), np.float32),
    }
    for t in range(T):
        shared[f"wlg{t}"] = np.broadcast_to(Wl[t, 0, :F], (128, F)).astype(NP_BF16)
        shared[f"wln{t}"] = np.broadcast_to(Wl[t, 0, F:], (128, F)).astype(NP_BF16)
        shared[f"wpt{t}"] = Wp[t].T.copy().astype(NP_BF16)
        shared[f"wiht{t}"] = Wih[t].T.copy().astype(NP_BF16)
        shared[f"whht{t}"] = Whh[t].T.copy().astype(NP_BF16)
        shared[f"brz{t}"] = np.broadcast_to(bih[t, :2 * F] + bhh[t, :2 * F], (128, 2 * F)).astype(np.float32).copy()
        shared[f"bin{t}"] = np.broadcast_to(bih[t, 2 * F:], (128, F)).astype(np.float32).copy()
        shared[f"bhn{t}"] = np.broadcast_to(bhh[t, 2 * F:], (128, F)).astype(np.float32).copy()
        shared[f"bpb{t}"] = np.broadcast_to(bp[t], (128, F)).astype(np.float32).copy()

    in_maps = []
    for c in range(NCORES):
        nf_pad, segrel = _prep_core(
            node_feats, seg, bounds_g[c], bounds_g[c + 1],
            bounds_n[c], bounds_n[c + 1], NT_G, NSUB)
        m = dict(shared)
        m["nf"] = nf_pad
        m["segrel"] = segrel
        in_maps.append(m)

    res = run_bass_kernel_spmd(nc, in_maps, core_ids=list(range(NCORES)))
    ctx.close()
    global LAST_RESULT
    LAST_RESULT = res

    out = np.zeros((G, F), np.float32)
    for c in range(NCORES):
        gc = bounds_g[c + 1] - bounds_g[c]
        out[bounds_g[c]:bounds_g[c + 1]] = res.results[c]["out"][:gc]
    return out



# revision 5
# speedup vs baseline: 1.7449x; 1.7449x over previous
"""AttentiveFP readout kernel for 8 Trainium2 NeuronCores (v2).

Graph-contiguous sharding of V=500k nodes across 8 cores (seg_ids
sorted, split at graph boundaries); every graph lives on one core so
all segment ops are core-local (no collectives).

v2 strategy vs v1: every F-contraction runs on the TensorEngine and
every per-node broadcast is a PE matmul; the Vector engine only does
cheap elementwise work.
 - node feats DMA'd once per tile in bf16, host-packed partition-major
   with the ones column baked in (d comes free from the matmul chain)
 - host also ships the one-hot membership matrix mn [node, graph] (for
   segment-sum matmuls) and its transpose mnt [graph, node] (used to
   broadcast per-graph attention logits u to nodes via N=1 matmuls)
 - per-node logit dot n_v = x . wln runs on PE against a host-side
   transposed copy of the features (nft)
 - leaky-relu via Act Prelu(alpha=0.01); single activation table
   (exp/tanh/relu): sigmoid(y) = (1+tanh(y/2))/2 with the GRU n-gate
   weights pre-scaled x2 on host; ELU's -1 folded into GRU bias rows
 - attention fold M' = mn * e split across DVE and GpSimd
"""

import numpy as np
from contextlib import ExitStack

import concourse.bass as bass
import concourse.bacc as bacc
import concourse.mybir as mybir
from concourse import tile
from concourse.bass_utils import run_bass_kernel_spmd

F32 = mybir.dt.float32
BF16 = mybir.dt.bfloat16
NP_BF16 = mybir.dt.np(mybir.dt.bfloat16)
AOP = mybir.AluOpType
ACT = mybir.ActivationFunctionType
AX = mybir.AxisListType

NCORES = 8
F = 256
T = 2
G = 25000
LAST_RESULT = None


def _build_program(NT_G, NSUB, bl_vals):
    ctx = ExitStack()
    nc = bacc.Bacc("TRN2")

    nfa_d = nc.dram_tensor("nfa", [128, NT_G, NSUB, F + 1], BF16, kind="ExternalInput")
    nft_d = nc.dram_tensor("nft", [128, NT_G, NSUB, 2, 128], BF16, kind="ExternalInput")
    mn_d = nc.dram_tensor("mn", [128, NT_G, NSUB, 128], BF16, kind="ExternalInput")
    mnt_d = nc.dram_tensor("mnt", [128, NT_G, NSUB, 128], BF16, kind="ExternalInput")
    ones1_d = nc.dram_tensor("ones1", [1, 128], BF16, kind="ExternalInput")
    identb_d = nc.dram_tensor("identb", [128, 128], BF16, kind="ExternalInput")
    wln2_d = nc.dram_tensor("wln2", [128, 2, T], BF16, kind="ExternalInput")
    wlg_d = nc.dram_tensor("wlg", [128, T, F], BF16, kind="ExternalInput")
    wpt_d = nc.dram_tensor("wpt", [128, T, 2, F], BF16, kind="ExternalInput")
    wih_d = nc.dram_tensor("wih", [128, T, 2, 3 * F], BF16, kind="ExternalInput")
    whh_d = nc.dram_tensor("whh", [128, T, 2, 3 * F], BF16, kind="ExternalInput")
    brz_d = nc.dram_tensor("brz", [1, T, 2 * F], BF16, kind="ExternalInput")
    bin2_d = nc.dram_tensor("bin2", [1, T, F], BF16, kind="ExternalInput")
    bhn_d = nc.dram_tensor("bhn", [1, T, F], BF16, kind="ExternalInput")
    bp_d = nc.dram_tensor("bp", [1, T, F], BF16, kind="ExternalInput")
    out_d = nc.dram_tensor("out", [NT_G * 128, F], F32, kind="ExternalOutput")

    SPL = NSUB // 2  # e-fold split point: [0,SPL) on DVE, [SPL,NSUB) gpsimd

    with tile.TileContext(nc) as tc:
      with tc.sbuf_pool(name="const", bufs=1) as cpool, \
           tc.sbuf_pool(name="nfa", bufs=2) as nfap, \
           tc.sbuf_pool(name="nft", bufs=2) as nftp, \
           tc.sbuf_pool(name="mn", bufs=2) as mnp, \
           tc.sbuf_pool(name="mnt", bufs=2) as mntp, \
           tc.sbuf_pool(name="mp", bufs=2) as mpp, \
           tc.sbuf_pool(name="gf", bufs=3) as gfp, \
           tc.sbuf_pool(name="wk", bufs=3) as wk, \
           tc.psum_pool(name="pzn", bufs=1) as pzn, \
           tc.psum_pool(name="pub", bufs=1) as pub, \
           tc.psum_pool(name="pw", bufs=2) as pw, \
           tc.psum_pool(name="prz", bufs=1) as prz, \
           tc.psum_pool(name="pnn", bufs=2) as pnn, \
           tc.psum_pool(name="pt", bufs=1) as pt:

        ones1 = cpool.tile_from(ones1_d[:, :], name="ones1")
        identb = cpool.tile_from(identb_d[:, :], name="identb")
        wln2 = cpool.tile_from(wln2_d[:, :, :], name="wln2")
        wlg = cpool.tile_from(wlg_d[:, :, :], name="wlg")
        wpt = cpool.tile_from(wpt_d[:, :, :, :], name="wpt")
        wih = cpool.tile_from(wih_d[:, :, :, :], name="wih")
        whh = cpool.tile_from(whh_d[:, :, :, :], name="whh")
        brz = cpool.tile_from(brz_d[:, :, :], name="brz")
        bin2 = cpool.tile_from(bin2_d[:, :, :], name="bin2")
        bhn = cpool.tile_from(bhn_d[:, :, :], name="bhn")
        bp = cpool.tile_from(bp_d[:, :, :], name="bp")

        for j in range(NT_G):
            nfa = nfap.tile([128, NSUB, F + 1], BF16, name=f"nfa{j}", tag="nfa")
            nc.sync.dma_start(nfa[:, :, :], nfa_d[:, j, :, :])
            nft = nftp.tile([128, NSUB, 2, 128], BF16, name=f"nft{j}", tag="nft")
            nc.sync.dma_start(nft[:, :, :, :], nft_d[:, j, :, :, :])
            mn = mnp.tile([128, NSUB, 128], BF16, name=f"mn{j}", tag="mn")
            nc.sync.dma_start(mn[:, :, :], mn_d[:, j, :, :])
            mnt = mntp.tile([128, NSUB, 128], BF16, name=f"mnt{j}", tag="mnt")
            nc.sync.dma_start(mnt[:, :, :], mnt_d[:, j, :, :])

            # init graph feats: psW0[g, 1:] = sum_n x  (col 0 = node count)
            psW0 = pw.tile([128, F + 1], F32, name=f"psW0_{j}", tag="pw")
            for s in range(NSUB):
                nc.tensor.matmul(psW0[:, :], mn[:, s, :], nfa[:, s, :],
                                 start=(s == 0), stop=(s == NSUB - 1))
            gf = gfp.tile([128, F], F32, name=f"gf0_{j}", tag="gf")
            nc.scalar.copy(gf[:, :], psW0[:, 1:F + 1])
            gfb = gfp.tile([128, F], BF16, name=f"gfb0_{j}", tag="gfb")
            nc.scalar.copy(gfb[:, :], psW0[:, 1:F + 1])

            # per-node logit dots for both t: zn[:, s, t] = x . wln[t]
            zn = pzn.tile([128, NSUB, T], F32, name=f"zn{j}", tag="zn")
            for s in range(NSUB):
                for k in range(2):
                    nc.tensor.matmul(zn[:, s, :], nft[:, s, k, :], wln2[:, k, :],
                                     start=(k == 0), stop=(k == 1))
            znsb = wk.tile([128, NSUB, T], F32, name=f"znsb{j}", tag="znsb")
            nc.scalar.copy(znsb[:, :, :], zn[:, :, :])

            for t in range(T):
                # u = wlg . relu(gf) + bl  (per graph)
                rgf = wk.tile([128, F], BF16, name=f"rgf{j}_{t}", tag="rgf")
                nc.scalar.activation(rgf[:, :], gf[:, :], ACT.Relu)
                uscr = wk.tile([128, F], BF16, name=f"uscr{j}_{t}", tag="uscr")
                nc.vector.tensor_tensor(uscr[:, :], rgf[:, :], wlg[:, t, :],
                                        op=AOP.mult)
                ucol = wk.tile([128, 1], F32, name=f"ucol{j}_{t}", tag="ucol")
                nc.vector.tensor_reduce(ucol[:, :], uscr[:, :], axis=AX.X, op=AOP.add)
                ucb = wk.tile([128, 1], BF16, name=f"ucb{j}_{t}", tag="ucb")
                nc.vector.tensor_scalar_add(ucb[:, :], ucol[:, :], float(bl_vals[t]))
                # broadcast u to node slots: ub[p, s] = sum_g mnt[g,s,p]*u[g]
                ub = pub.tile([128, NSUB], F32, name=f"ub{j}_{t}", tag="ub")
                for s in range(NSUB):
                    nc.tensor.matmul(ub[:, s:s + 1], mnt[:, s, :], ucb[:, :],
                                     start=True, stop=True)
                # z = n + u ; e = exp(leaky_relu(z))
                zs = wk.tile([128, NSUB], F32, name=f"zs{j}_{t}", tag="zs")
                nc.vector.tensor_tensor(zs[:, :], znsb[:, :, t], ub[:, :], op=AOP.add)
                zl = wk.tile([128, NSUB], F32, name=f"zl{j}_{t}", tag="zl")
                nc.scalar.activation(zl[:, :], zs[:, :], ACT.Prelu, alpha=0.01)
                ebf = wk.tile([128, NSUB], BF16, name=f"ebf{j}_{t}", tag="ebf")
                nc.scalar.activation(ebf[:, :], zl[:, :], ACT.Exp)
                # M' = mn * e  (split DVE / gpsimd)
                mp = mpp.tile([128, NSUB, 128], BF16, name=f"mp{j}_{t}", tag="mp")
                nc.vector.tensor_tensor(
                    mp[:, 0:SPL, :], mn[:, 0:SPL, :],
                    ebf[:, 0:SPL].unsqueeze(2).broadcast_to((128, SPL, 128)),
                    op=AOP.mult)
                nc.gpsimd.tensor_tensor(
                    mp[:, SPL:NSUB, :], mn[:, SPL:NSUB, :],
                    ebf[:, SPL:NSUB].unsqueeze(2).broadcast_to((128, NSUB - SPL, 128)),
                    op=AOP.mult)
                # weighted segment sum: psW = [d | sum e*x]
                psW = pw.tile([128, F + 1], F32, name=f"psW{j}_{t}", tag="pw")
                for s in range(NSUB):
                    nc.tensor.matmul(psW[:, :], mp[:, s, :], nfa[:, s, :],
                                     start=(s == 0), stop=(s == NSUB - 1))
                dmx = wk.tile([128, 1], F32, name=f"dmx{j}_{t}", tag="dmx")
                nc.vector.tensor_scalar_max(dmx[:, :], psW[:, 0:1], 1e-30)
                recd = wk.tile([128, 1], F32, name=f"recd{j}_{t}", tag="recd")
                nc.vector.reciprocal(recd[:, :], dmx[:, :])
                stl = wk.tile([128, F], BF16, name=f"stl{j}_{t}", tag="stl")
                nc.vector.tensor_scalar(stl[:, :], psW[:, 1:F + 1], recd[:, :],
                                        None, op0=AOP.mult)
                # g_repr = stl @ Wp[t].T + bp[t]  (via stlT chunks)
                stlT = wk.tile([128, 2, 128], BF16, name=f"stlT{j}_{t}", tag="stlT")
                for k in range(2):
                    ptt = pt.tile([128, 128], BF16, name=f"ptt{j}_{t}_{k}", tag="pt")
                    nc.tensor.transpose(ptt[:, :], stl[:, k * 128:(k + 1) * 128],
                                        identb[:, :])
                    nc.scalar.copy(stlT[:, k, :], ptt[:, :])
                pwp = pnn.tile([128, F], F32, name=f"pwp{j}_{t}", tag="pnn")
                nc.tensor.matmul(pwp[:, :], ones1[:, :], bp[:, t, :],
                                 start=True, stop=False)
                for k in range(2):
                    nc.tensor.matmul(pwp[:, :], stlT[:, k, :], wpt[:, t, k, :],
                                     start=False, stop=(k == 1))
                # ctxp1 = elu(g_repr)+1 = relu(x) + exp(min(x,0))
                xn = wk.tile([128, F], F32, name=f"xn{j}_{t}", tag="xn")
                nc.vector.tensor_scalar_min(xn[:, :], pwp[:, :], 0.0)
                en = wk.tile([128, F], F32, name=f"en{j}_{t}", tag="en")
                nc.scalar.activation(en[:, :], xn[:, :], ACT.Exp)
                xp = wk.tile([128, F], F32, name=f"xp{j}_{t}", tag="xp")
                nc.scalar.activation(xp[:, :], pwp[:, :], ACT.Relu)
                ctxp1 = wk.tile([128, F], BF16, name=f"ctx{j}_{t}", tag="ctx")
                nc.vector.tensor_tensor(ctxp1[:, :], en[:, :], xp[:, :], op=AOP.add)
                # GRU: transposed operands
                ctxT = wk.tile([128, 2, 128], BF16, name=f"ctxT{j}_{t}", tag="ctxT")
                hT = wk.tile([128, 2, 128], BF16, name=f"hT{j}_{t}", tag="hT")
                for k in range(2):
                    p1 = pt.tile([128, 128], BF16, name=f"p1{j}_{t}_{k}", tag="pt")
                    nc.tensor.transpose(p1[:, :], ctxp1[:, k * 128:(k + 1) * 128],
                                        identb[:, :])
                    nc.scalar.copy(ctxT[:, k, :], p1[:, :])
                    p2 = pt.tile([128, 128], BF16, name=f"p2{j}_{t}_{k}", tag="pt")
                    nc.tensor.transpose(p2[:, :], gfb[:, k * 128:(k + 1) * 128],
                                        identb[:, :])
                    nc.scalar.copy(hT[:, k, :], p2[:, :])
                # gates: rz = sum of x/h parts + bias (bias via K=1 matmul)
                ps_rz = prz.tile([128, 2 * F], F32, name=f"psrz{j}_{t}", tag="prz")
                nc.tensor.matmul(ps_rz[:, :], ones1[:, :], brz[:, t, :],
                                 start=True, stop=False)
                mm = 0
                for lhsT, wt in ((ctxT, wih), (hT, whh)):
                    for k in range(2):
                        nc.tensor.matmul(ps_rz[:, :], lhsT[:, k, :],
                                         wt[:, t, k, 0:2 * F],
                                         start=False, stop=(mm == 3))
                        mm += 1
                trz = wk.tile([128, 2 * F], BF16, name=f"trz{j}_{t}", tag="trz")
                nc.scalar.activation(trz[:, :], ps_rz[:, :], ACT.Tanh, scale=0.5)
                ps_in = pnn.tile([128, F], F32, name=f"psin{j}_{t}", tag="pnn")
                nc.tensor.matmul(ps_in[:, :], ones1[:, :], bin2[:, t, :],
                                 start=True, stop=False)
                for k in range(2):
                    nc.tensor.matmul(ps_in[:, :], ctxT[:, k, :],
                                     wih[:, t, k, 2 * F:3 * F],
                                     start=False, stop=(k == 1))
                ps_hn = pnn.tile([128, F], F32, name=f"pshn{j}_{t}", tag="pnn")
                nc.tensor.matmul(ps_hn[:, :], ones1[:, :], bhn[:, t, :],
                                 start=True, stop=False)
                for k in range(2):
                    nc.tensor.matmul(ps_hn[:, :], hT[:, k, :],
                                     whh[:, t, k, 2 * F:3 * F],
                                     start=False, stop=(k == 1))
                # nn = tanh(inn + bin + r*hn), r = (1+tanh(rz/2))/2
                av = wk.tile([128, F], F32, name=f"av{j}_{t}", tag="av")
                nc.vector.tensor_tensor(av[:, :], trz[:, 0:F], ps_hn[:, :],
                                        op=AOP.mult)
                bv = wk.tile([128, F], F32, name=f"bv{j}_{t}", tag="bv")
                nc.vector.tensor_tensor(bv[:, :], av[:, :], ps_hn[:, :], op=AOP.add)
                cv = wk.tile([128, F], F32, name=f"cv{j}_{t}", tag="cv")
                nc.vector.tensor_tensor(cv[:, :], bv[:, :], ps_in[:, :], op=AOP.add)
                nn = wk.tile([128, F], F32, name=f"nn{j}_{t}", tag="nn")
                nc.scalar.activation(nn[:, :], cv[:, :], ACT.Tanh, scale=0.5)
                # h' = nn + 0.5*(1+tanh(z/2))*(h-nn)
                hm = wk.tile([128, F], F32, name=f"hm{j}_{t}", tag="hm")
                nc.vector.tensor_tensor(hm[:, :], gf[:, :], nn[:, :], op=AOP.subtract)
                qv = wk.tile([128, F], F32, name=f"qv{j}_{t}", tag="qv")
                nc.vector.tensor_tensor(qv[:, :], trz[:, F:2 * F], hm[:, :],
                                        op=AOP.mult)
                h2 = wk.tile([128, F], F32, name=f"h2{j}_{t}", tag="h2")
                nc.vector.tensor_tensor(h2[:, :], hm[:, :], qv[:, :], op=AOP.add)
                h3 = wk.tile([128, F], F32, name=f"h3{j}_{t}", tag="h3")
                nc.vector.tensor_scalar_mul(h3[:, :], h2[:, :], 0.5)
                gf_new = gfp.tile([128, F], F32, name=f"gf{j}_{t}", tag="gf")
                nc.vector.tensor_tensor(gf_new[:, :], nn[:, :], h3[:, :], op=AOP.add)
                gf = gf_new
                if t == 0:
                    gfb = gfp.tile([128, F], BF16, name=f"gfb{j}_{t}", tag="gfb")
                    nc.scalar.copy(gfb[:, :], gf[:, :])
            nc.sync.dma_start(out_d[j * 128:(j + 1) * 128, :], gf[:, :])
    nc.finalize()
    return nc, ctx


def _prep_core(node_feats, seg, g_lo, g_hi, NT_G, NSUB):
    """Build packed per-core arrays: nfa, nft, mn, mnt."""
    nfa = np.zeros((128, NT_G, NSUB, F + 1), NP_BF16)
    nft = np.zeros((128, NT_G, NSUB, 2, 128), NP_BF16)
    mn = np.zeros((128, NT_G, NSUB, 128), NP_BF16)
    mnt = np.zeros((128, NT_G, NSUB, 128), NP_BF16)
    gidx = np.arange(128, dtype=np.int32)
    for j in range(NT_G):
        gt = g_lo + j * 128
        if gt >= g_hi:
            continue
        ge = min(gt + 128, g_hi)
        a = int(np.searchsorted(seg, gt, 'left'))
        b = int(np.searchsorted(seg, ge, 'left'))
        cnt = b - a
        x = np.zeros((NSUB * 128, F), np.float32)
        x[:cnt] = node_feats[a:b]
        rel = np.full(NSUB * 128, -1, np.int32)
        rel[:cnt] = seg[a:b] - gt
        xc = x.reshape(NSUB, 128, F)
        # nfa[p, j, s, 0]=valid, [.., 1+f]=x
        nfa[:, j, :, 0] = (rel.reshape(NSUB, 128) >= 0).T.astype(NP_BF16)
        nfa[:, j, :, 1:] = xc.transpose(1, 0, 2).astype(NP_BF16)
        # nft[fp, j, s, k, p] = x[node(s,p), k*128+fp]
        nft[:, j] = xc.reshape(NSUB, 128, 2, 128).transpose(3, 0, 2, 1).astype(NP_BF16)
        oh = (rel.reshape(NSUB, 128)[:, :, None] == gidx[None, None, :])  # [s,p,g]
        mn[:, j] = oh.transpose(1, 0, 2).astype(NP_BF16)   # [p, s, g]
        mnt[:, j] = oh.transpose(2, 0, 1).astype(NP_BF16)  # [g, s, p]
    return nfa, nft, mn, mnt


def kernel(node_feats, seg_ids, Wl, bl, Wp, bp, Wih, Whh, bih, bhh):
    node_feats = np.asarray(node_feats, np.float32)
    seg = np.asarray(seg_ids).astype(np.int64)
    Wl = np.asarray(Wl, np.float32)
    bl = np.asarray(bl, np.float32)
    Wp = np.asarray(Wp, np.float32)
    bp = np.asarray(bp, np.float32)
    Wih = np.asarray(Wih, np.float32)
    Whh = np.asarray(Whh, np.float32)
    bih = np.asarray(bih, np.float32)
    bhh = np.asarray(bhh, np.float32)
    V = node_feats.shape[0]

    bounds_g = [0]
    for c in range(1, NCORES):
        bounds_g.append(int(seg[c * V // NCORES]))
    bounds_g.append(G)

    NT_G = max((bounds_g[c + 1] - bounds_g[c] + 127) // 128 for c in range(NCORES))
    maxnodes = 1
    for c in range(NCORES):
        for gt in range(bounds_g[c], bounds_g[c + 1], 128):
            ge = min(gt + 128, bounds_g[c + 1])
            a = np.searchsorted(seg, gt, 'left')
            b = np.searchsorted(seg, ge, 'left')
            maxnodes = max(maxnodes, int(b - a))
    NSUB = (maxnodes + 127) // 128

    nc, ctx = _build_program(NT_G, NSUB, [float(bl[t, 0]) for t in range(T)])

    # replicated weight arrays
    fr = np.arange(128)
    wln2 = np.zeros((128, 2, T), np.float32)
    for t in range(T):
        for k in range(2):
            wln2[:, k, t] = Wl[t, 0, F + k * 128:F + (k + 1) * 128]
    wlg = np.zeros((128, T, F), np.float32)
    wpt = np.zeros((128, T, 2, F), np.float32)
    wih = np.zeros((128, T, 2, 3 * F), np.float32)
    whh = np.zeros((128, T, 2, 3 * F), np.float32)
    brz = np.zeros((1, T, 2 * F), np.float32)
    bin2 = np.zeros((1, T, F), np.float32)
    bhn = np.zeros((1, T, F), np.float32)
    bpr = np.zeros((1, T, F), np.float32)
    for t in range(T):
        wlg[:, t, :] = np.broadcast_to(Wl[t, 0, :F], (128, F))
        for k in range(2):
            wpt[:, t, k, :] = Wp[t][:, k * 128:(k + 1) * 128].T
            wih[:, t, k, :] = Wih[t][:, k * 128:(k + 1) * 128].T
            whh[:, t, k, :] = Whh[t][:, k * 128:(k + 1) * 128].T
        # n-gate input half pre-scaled x2 for the tanh(x/2) sigmoid trick
        wih[:, t, :, 2 * F:] *= 2.0
        # ctx is fed as ctx+1; subtract column sums of Wih from biases
        csum = Wih[t].sum(axis=1)  # [3F]
        brz[0, t, :] = bih[t, :2 * F] + bhh[t, :2 * F] - csum[:2 * F]
        bin2[0, t, :] = 2.0 * (bih[t, 2 * F:] - csum[2 * F:])
        bhn[0, t, :] = bhh[t, 2 * F:]
        bpr[0, t, :] = bp[t]
    shared = {
        "ones1": np.ones((1, 128), np.float32).astype(NP_BF16),
        "identb": np.eye(128, dtype=np.float32).astype(NP_BF16),
        "wln2": wln2.astype(NP_BF16), "wlg": wlg.astype(NP_BF16),
        "wpt": wpt.astype(NP_BF16), "wih": wih.astype(NP_BF16),
        "whh": whh.astype(NP_BF16), "brz": brz.astype(NP_BF16),
        "bin2": bin2.astype(NP_BF16), "bhn": bhn.astype(NP_BF16),
        "bp": bpr.astype(NP_BF16),
    }

    in_maps = []
    for c in range(NCORES):
        nfa, nft, mn, mnt = _prep_core(
            node_feats, seg, bounds_g[c], bounds_g[c + 1], NT_G, NSUB)
        m = dict(shared)
        m["nfa"] = nfa
        m["nft"] = nft
        m["mn"] = mn
        m["mnt"] = mnt
        in_maps.append(m)

    res = run_bass_kernel_spmd(nc, in_maps, core_ids=list(range(NCORES)))
    ctx.close()
    global LAST_RESULT
    LAST_RESULT = res

    out = np.zeros((G, F), np.float32)
    for c in range(NCORES):
        gc = bounds_g[c + 1] - bounds_g[c]
        out[bounds_g[c]:bounds_g[c + 1]] = res.results[c]["out"][:gc]
    return out


# revision 11
# speedup vs baseline: 2.3751x; 1.3612x over previous
"""AttentiveFP readout kernel for 8 Trainium2 NeuronCores (v2).

Graph-contiguous sharding of V=500k nodes across 8 cores (seg_ids
sorted, split at graph boundaries); every graph lives on one core so
all segment ops are core-local (no collectives).

v2 strategy vs v1: every F-contraction runs on the TensorEngine and
every per-node broadcast is a PE matmul; the Vector engine only does
cheap elementwise work.
 - node feats DMA'd once per tile in bf16, host-packed partition-major
   with the ones column baked in (d comes free from the matmul chain)
 - host also ships the one-hot membership matrix mn [node, graph] (for
   segment-sum matmuls) and its transpose mnt [graph, node] (used to
   broadcast per-graph attention logits u to nodes via N=1 matmuls)
 - per-node logit dot n_v = x . wln runs on PE against a host-side
   transposed copy of the features (nft)
 - leaky-relu via Act Prelu(alpha=0.01); single activation table
   (exp/tanh/relu): sigmoid(y) = (1+tanh(y/2))/2 with the GRU n-gate
   weights pre-scaled x2 on host; ELU's -1 folded into GRU bias rows
 - attention fold M' = mn * e split across DVE and GpSimd
"""

import numpy as np
from contextlib import ExitStack

import concourse.bass as bass
import concourse.bacc as bacc
import concourse.mybir as mybir
from concourse import tile
from concourse.bass_utils import run_bass_kernel_spmd

F32 = mybir.dt.float32
BF16 = mybir.dt.bfloat16
NP_BF16 = mybir.dt.np(mybir.dt.bfloat16)
AOP = mybir.AluOpType
ACT = mybir.ActivationFunctionType
AX = mybir.AxisListType

NCORES = 8
F = 256
T = 2
G = 25000
LAST_RESULT = None


def _build_program(NT_G, NSUB, bl_vals):
    ctx = ExitStack()
    nc = bacc.Bacc("TRN2")

    nfa_d = nc.dram_tensor("nfa", [128, NT_G, NSUB, F + 1], BF16, kind="ExternalInput")
    nft_d = nc.dram_tensor("nft", [128, NT_G, NSUB, 2, 128], BF16, kind="ExternalInput")
    mn_d = nc.dram_tensor("mn", [128, NT_G, NSUB, 128], BF16, kind="ExternalInput")
    mnt_d = nc.dram_tensor("mnt", [128, NT_G, NSUB, 128], BF16, kind="ExternalInput")
    ones1_d = nc.dram_tensor("ones1", [1, 128], BF16, kind="ExternalInput")
    identb_d = nc.dram_tensor("identb", [128, 128], BF16, kind="ExternalInput")
    wln2_d = nc.dram_tensor("wln2", [128, 2, T], BF16, kind="ExternalInput")
    wlg_d = nc.dram_tensor("wlg", [128, T, F], BF16, kind="ExternalInput")
    wpt_d = nc.dram_tensor("wpt", [128, T, 2, F], BF16, kind="ExternalInput")
    wih_d = nc.dram_tensor("wih", [128, T, 2, 3 * F], BF16, kind="ExternalInput")
    whh_d = nc.dram_tensor("whh", [128, T, 2, 3 * F], BF16, kind="ExternalInput")
    brz_d = nc.dram_tensor("brz", [1, T, 2 * F], BF16, kind="ExternalInput")
    bin2_d = nc.dram_tensor("bin2", [1, T, F], BF16, kind="ExternalInput")
    bhn_d = nc.dram_tensor("bhn", [1, T, F], BF16, kind="ExternalInput")
    bp_d = nc.dram_tensor("bp", [1, T, F], BF16, kind="ExternalInput")
    out_d = nc.dram_tensor("out", [NT_G * 128, F], F32, kind="ExternalOutput")

    # e-fold split: DVE is ~2.6x faster per element than gpsimd
    SPL = min(NSUB, max(1, (NSUB * 5 + 3) // 7))

    with tile.TileContext(nc) as tc:
      with tc.sbuf_pool(name="const", bufs=1) as cpool, \
           tc.sbuf_pool(name="nfa", bufs=3) as nfap, \
           tc.sbuf_pool(name="nft", bufs=3) as nftp, \
           tc.sbuf_pool(name="mn", bufs=3) as mnp, \
           tc.sbuf_pool(name="mnt", bufs=3) as mntp, \
           tc.sbuf_pool(name="mp", bufs=2) as mpp, \
           tc.sbuf_pool(name="gf", bufs=3) as gfp, \
           tc.sbuf_pool(name="wk", bufs=3) as wk, \
           tc.psum_pool(name="pzn", bufs=1) as pzn, \
           tc.psum_pool(name="pub", bufs=1) as pub, \
           tc.psum_pool(name="pw", bufs=2) as pw, \
           tc.psum_pool(name="prz", bufs=1) as prz, \
           tc.psum_pool(name="pnn", bufs=2) as pnn, \
           tc.psum_pool(name="pt", bufs=1) as pt:

        ones1 = cpool.tile_from(ones1_d[:, :], name="ones1")
        identb = cpool.tile_from(identb_d[:, :], name="identb")
        wln2 = cpool.tile_from(wln2_d[:, :, :], name="wln2")
        wlg = cpool.tile_from(wlg_d[:, :, :], name="wlg")
        wpt = cpool.tile_from(wpt_d[:, :, :, :], name="wpt")
        wih = cpool.tile_from(wih_d[:, :, :, :], name="wih")
        whh = cpool.tile_from(whh_d[:, :, :, :], name="whh")
        brz = cpool.tile_from(brz_d[:, :, :], name="brz")
        bin2 = cpool.tile_from(bin2_d[:, :, :], name="bin2")
        bhn = cpool.tile_from(bhn_d[:, :, :], name="bhn")
        bp = cpool.tile_from(bp_d[:, :, :], name="bp")

        st = {}

        def emit_dma(j):
            if j >= NT_G:
                return
            d = {}
            d["nfa"] = nfap.tile([128, NSUB, F + 1], BF16, name=f"nfa{j}", tag="nfa")
            nc.sync.dma_start(d["nfa"][:, :, :], nfa_d[:, j, :, :])
            d["nft"] = nftp.tile([128, NSUB, 2, 128], BF16, name=f"nft{j}", tag="nft")
            nc.sync.dma_start(d["nft"][:, :, :, :], nft_d[:, j, :, :, :])
            d["mn"] = mnp.tile([128, NSUB, 128], BF16, name=f"mn{j}", tag="mn")
            nc.sync.dma_start(d["mn"][:, :, :], mn_d[:, j, :, :])
            d["mnt"] = mntp.tile([128, NSUB, 128], BF16, name=f"mnt{j}", tag="mnt")
            nc.sync.dma_start(d["mnt"][:, :, :], mnt_d[:, j, :, :])
            st[j] = d

        def emit_init(j):
            # init graph feats: psW0[g, 1:] = sum_n x (col 0 = node count)
            if j >= NT_G:
                return
            d = st[j]
            psW0 = pw.tile([128, F + 1], F32, name=f"psW0_{j}", tag="pw")
            for s in range(NSUB):
                nc.tensor.matmul(psW0[:, :], d["mn"][:, s, :], d["nfa"][:, s, :],
                                 start=(s == 0), stop=(s == NSUB - 1))
            d["psW0"] = psW0

        def emit_gf(j):
            if j >= NT_G:
                return
            d = st[j]
            d["gf"] = gfp.tile([128, F], F32, name=f"gf0_{j}", tag="gf", bufs=4)
            nc.scalar.copy(d["gf"][:, :], d["psW0"][:, 1:F + 1])
            d["gfb"] = gfp.tile([128, F], BF16, name=f"gfb0_{j}", tag="gfb")
            nc.scalar.copy(d["gfb"][:, :], d["psW0"][:, 1:F + 1])

        def emit_zdot(j):
            # per-node logit dots for both t: zn[:, s, t] = x . wln[t]
            if j >= NT_G:
                return
            d = st[j]
            zn = pzn.tile([128, NSUB, T], F32, name=f"zn{j}", tag="zn")
            for s in range(NSUB):
                for k in range(2):
                    nc.tensor.matmul(zn[:, s, :], d["nft"][:, s, k, :],
                                     wln2[:, k, :], start=(k == 0), stop=(k == 1))
            d["znsb"] = wk.tile([128, NSUB, T], F32, name=f"znsb{j}", tag="znsb")
            nc.scalar.copy(d["znsb"][:, :, :], zn[:, :, :])

        emit_dma(0)
        emit_dma(1)
        emit_init(0)
        emit_gf(0)
        emit_zdot(0)

        for j in range(NT_G):
            d = st[j]
            nfa, mn, mnt, znsb = d["nfa"], d["mn"], d["mnt"], d["znsb"]
            gf, gfb = d["gf"], d["gfb"]
            emit_dma(j + 2)
            for t in range(T):
                # u = wlg . relu(gf) + bl  (per graph)
                rgf = wk.tile([128, F], BF16, name=f"rgf{j}_{t}", tag="rgf")
                nc.scalar.activation(rgf[:, :], gf[:, :], ACT.Relu)
                uscr = wk.tile([128, F], BF16, name=f"uscr{j}_{t}", tag="uscr")
                nc.vector.tensor_tensor(uscr[:, :], rgf[:, :], wlg[:, t, :],
                                        op=AOP.mult)
                ucol = wk.tile([128, 1], F32, name=f"ucol{j}_{t}", tag="ucol")
                nc.vector.tensor_reduce(ucol[:, :], uscr[:, :], axis=AX.X, op=AOP.add)
                ucb = wk.tile([128, 1], BF16, name=f"ucb{j}_{t}", tag="ucb")
                nc.vector.tensor_scalar_add(ucb[:, :], ucol[:, :], float(bl_vals[t]))
                # broadcast u to node slots: ub[p, s] = sum_g mnt[g,s,p]*u[g]
                ub = pub.tile([128, NSUB], F32, name=f"ub{j}_{t}", tag="ub")
                for s in range(NSUB):
                    nc.tensor.matmul(ub[:, s:s + 1], mnt[:, s, :], ucb[:, :],
                                     start=True, stop=True)
                # z = n + u ; e = exp(leaky_relu(z))
                zs = wk.tile([128, NSUB], F32, name=f"zs{j}_{t}", tag="zs")
                nc.vector.tensor_tensor(zs[:, :], znsb[:, :, t], ub[:, :], op=AOP.add)
                zl = wk.tile([128, NSUB], F32, name=f"zl{j}_{t}", tag="zl")
                nc.scalar.activation(zl[:, :], zs[:, :], ACT.Prelu, alpha=0.01)
                ebf = wk.tile([128, NSUB], BF16, name=f"ebf{j}_{t}", tag="ebf")
                nc.scalar.activation(ebf[:, :], zl[:, :], ACT.Exp)
                # M' = mn * e  (split DVE / gpsimd)
                mp = mpp.tile([128, NSUB, 128], BF16, name=f"mp{j}_{t}", tag="mp")
                nc.vector.tensor_tensor(
                    mp[:, 0:SPL, :], mn[:, 0:SPL, :],
                    ebf[:, 0:SPL].unsqueeze(2).broadcast_to((128, SPL, 128)),
                    op=AOP.mult)
                nc.gpsimd.tensor_tensor(
                    mp[:, SPL:NSUB, :], mn[:, SPL:NSUB, :],
                    ebf[:, SPL:NSUB].unsqueeze(2).broadcast_to((128, NSUB - SPL, 128)),
                    op=AOP.mult)
                if t == 0:
                    # fill the PE stall (waiting on e-fold) with j+1's init
                    emit_init(j + 1)
                # weighted segment sum: psW = [d | sum e*x]
                psW = pw.tile([128, F + 1], F32, name=f"psW{j}_{t}", tag="pw")
                for s in range(NSUB):
                    nc.tensor.matmul(psW[:, :], mp[:, s, :], nfa[:, s, :],
                                     start=(s == 0), stop=(s == NSUB - 1))
                dmx = wk.tile([128, 1], F32, name=f"dmx{j}_{t}", tag="dmx")
                nc.vector.tensor_scalar_max(dmx[:, :], psW[:, 0:1], 1e-30)
                recd = wk.tile([128, 1], F32, name=f"recd{j}_{t}", tag="recd")
                nc.vector.reciprocal(recd[:, :], dmx[:, :])
                stl = wk.tile([128, F], BF16, name=f"stl{j}_{t}", tag="stl")
                nc.vector.tensor_scalar(stl[:, :], psW[:, 1:F + 1], recd[:, :],
                                        None, op0=AOP.mult)
                if t == 0:
                    emit_gf(j + 1)
                # g_repr = stl @ Wp[t].T + bp[t]  (via stlT chunks)
                stlT = wk.tile([128, 2, 128], BF16, name=f"stlT{j}_{t}", tag="stlT")
                for k in range(2):
                    ptt = pt.tile([128, 128], BF16, name=f"ptt{j}_{t}_{k}", tag="pt")
                    nc.tensor.transpose(ptt[:, :], stl[:, k * 128:(k + 1) * 128],
                                        identb[:, :])
                    nc.scalar.copy(stlT[:, k, :], ptt[:, :])
                pwp = pnn.tile([128, F], F32, name=f"pwp{j}_{t}", tag="pnn")
                nc.tensor.matmul(pwp[:, :], ones1[:, :], bp[:, t, :],
                                 start=True, stop=False)
                for k in range(2):
                    nc.tensor.matmul(pwp[:, :], stlT[:, k, :], wpt[:, t, k, :],
                                     start=False, stop=(k == 1))
                # ctxp1 = elu(g_repr)+1 = relu(x) + exp(min(x,0))
                xn = wk.tile([128, F], F32, name=f"xn{j}_{t}", tag="xn")
                nc.vector.tensor_scalar_min(xn[:, :], pwp[:, :], 0.0)
                en = wk.tile([128, F], F32, name=f"en{j}_{t}", tag="en")
                nc.scalar.activation(en[:, :], xn[:, :], ACT.Exp)
                xp = wk.tile([128, F], F32, name=f"xp{j}_{t}", tag="xp")
                nc.scalar.activation(xp[:, :], pwp[:, :], ACT.Relu)
                ctxp1 = wk.tile([128, F], BF16, name=f"ctx{j}_{t}", tag="ctx")
                nc.vector.tensor_tensor(ctxp1[:, :], en[:, :], xp[:, :], op=AOP.add)
                # GRU: transposed operands
                ctxT = wk.tile([128, 2, 128], BF16, name=f"ctxT{j}_{t}", tag="ctxT")
                hT = wk.tile([128, 2, 128], BF16, name=f"hT{j}_{t}", tag="hT")
                for k in range(2):
                    p1 = pt.tile([128, 128], BF16, name=f"p1{j}_{t}_{k}", tag="pt")
                    nc.tensor.transpose(p1[:, :], ctxp1[:, k * 128:(k + 1) * 128],
                                        identb[:, :])
                    nc.scalar.copy(ctxT[:, k, :], p1[:, :])
                    p2 = pt.tile([128, 128], BF16, name=f"p2{j}_{t}_{k}", tag="pt")
                    nc.tensor.transpose(p2[:, :], gfb[:, k * 128:(k + 1) * 128],
                                        identb[:, :])
                    nc.scalar.copy(hT[:, k, :], p2[:, :])
                # gates: rz = sum of x/h parts + bias (bias via K=1 matmul)
                ps_rz = prz.tile([128, 2 * F], F32, name=f"psrz{j}_{t}", tag="prz")
                nc.tensor.matmul(ps_rz[:, :], ones1[:, :], brz[:, t, :],
                                 start=True, stop=False)
                mm = 0
                for lhsT, wt in ((ctxT, wih), (hT, whh)):
                    for k in range(2):
                        nc.tensor.matmul(ps_rz[:, :], lhsT[:, k, :],
                                         wt[:, t, k, 0:2 * F],
                                         start=False, stop=(mm == 3))
                        mm += 1
                trz = wk.tile([128, 2 * F], BF16, name=f"trz{j}_{t}", tag="trz")
                nc.scalar.activation(trz[:, :], ps_rz[:, :], ACT.Tanh, scale=0.5)
                ps_in = pnn.tile([128, F], F32, name=f"psin{j}_{t}", tag="pnn")
                nc.tensor.matmul(ps_in[:, :], ones1[:, :], bin2[:, t, :],
                                 start=True, stop=False)
                for k in range(2):
                    nc.tensor.matmul(ps_in[:, :], ctxT[:, k, :],
                                     wih[:, t, k, 2 * F:3 * F],
                                     start=False, stop=(k == 1))
                ps_hn = pnn.tile([128, F], F32, name=f"pshn{j}_{t}", tag="pnn")
                nc.tensor.matmul(ps_hn[:, :], ones1[:, :], bhn[:, t, :],
                                 start=True, stop=False)
                for k in range(2):
                    nc.tensor.matmul(ps_hn[:, :], hT[:, k, :],
                                     whh[:, t, k, 2 * F:3 * F],
                                     start=False, stop=(k == 1))
                # nn = tanh(inn + bin + r*hn), r = (1+tanh(rz/2))/2
                av = wk.tile([128, F], F32, name=f"av{j}_{t}", tag="av")
                nc.vector.tensor_tensor(av[:, :], trz[:, 0:F], ps_hn[:, :],
                                        op=AOP.mult)
                bv = wk.tile([128, F], F32, name=f"bv{j}_{t}", tag="bv")
                nc.vector.tensor_tensor(bv[:, :], av[:, :], ps_hn[:, :], op=AOP.add)
                cv = wk.tile([128, F], F32, name=f"cv{j}_{t}", tag="cv")
                nc.vector.tensor_tensor(cv[:, :], bv[:, :], ps_in[:, :], op=AOP.add)
                nn = wk.tile([128, F], F32, name=f"nn{j}_{t}", tag="nn")
                nc.scalar.activation(nn[:, :], cv[:, :], ACT.Tanh, scale=0.5)
                # h' = nn + 0.5*(1+tanh(z/2))*(h-nn)
                hm = wk.tile([128, F], F32, name=f"hm{j}_{t}", tag="hm")
                nc.vector.tensor_tensor(hm[:, :], gf[:, :], nn[:, :], op=AOP.subtract)
                qv = wk.tile([128, F], F32, name=f"qv{j}_{t}", tag="qv")
                nc.vector.tensor_tensor(qv[:, :], trz[:, F:2 * F], hm[:, :],
                                        op=AOP.mult)
                h2 = wk.tile([128, F], F32, name=f"h2{j}_{t}", tag="h2")
                nc.vector.tensor_tensor(h2[:, :], hm[:, :], qv[:, :], op=AOP.add)
                h3 = wk.tile([128, F], F32, name=f"h3{j}_{t}", tag="h3")
                nc.vector.tensor_scalar_mul(h3[:, :], h2[:, :], 0.5)
                gf_new = gfp.tile([128, F], F32, name=f"gf{j}_{t}", tag="gf", bufs=4)
                nc.vector.tensor_tensor(gf_new[:, :], nn[:, :], h3[:, :], op=AOP.add)
                gf = gf_new
                if t == 0:
                    gfb = gfp.tile([128, F], BF16, name=f"gfb{j}_{t}", tag="gfb")
                    nc.scalar.copy(gfb[:, :], gf[:, :])
                    # fill the PE stall before t=1's u-dot with j+1's z-dots
                    emit_zdot(j + 1)
            nc.sync.dma_start(out_d[j * 128:(j + 1) * 128, :], gf[:, :])
    nc.finalize()
    return nc, ctx


def _prep_core(node_feats, seg, g_lo, g_hi, NT_G, NSUB):
    """Build packed per-core arrays: nfa, nft, mn, mnt."""
    nfa = np.zeros((128, NT_G, NSUB, F + 1), NP_BF16)
    nft = np.zeros((128, NT_G, NSUB, 2, 128), NP_BF16)
    mn = np.zeros((128, NT_G, NSUB, 128), NP_BF16)
    mnt = np.zeros((128, NT_G, NSUB, 128), NP_BF16)
    gidx = np.arange(128, dtype=np.int32)
    for j in range(NT_G):
        gt = g_lo + j * 128
        if gt >= g_hi:
            continue
        ge = min(gt + 128, g_hi)
        a = int(np.searchsorted(seg, gt, 'left'))
        b = int(np.searchsorted(seg, ge, 'left'))
        cnt = b - a
        x = np.zeros((NSUB * 128, F), np.float32)
        x[:cnt] = node_feats[a:b]
        rel = np.full(NSUB * 128, -1, np.int32)
        rel[:cnt] = seg[a:b] - gt
        xc = x.reshape(NSUB, 128, F)
        # nfa[p, j, s, 0]=valid, [.., 1+f]=x
        nfa[:, j, :, 0] = (rel.reshape(NSUB, 128) >= 0).T.astype(NP_BF16)
        nfa[:, j, :, 1:] = xc.transpose(1, 0, 2).astype(NP_BF16)
        # nft[fp, j, s, k, p] = x[node(s,p), k*128+fp]
        nft[:, j] = xc.reshape(NSUB, 128, 2, 128).transpose(3, 0, 2, 1).astype(NP_BF16)
        oh = (rel.reshape(NSUB, 128)[:, :, None] == gidx[None, None, :])  # [s,p,g]
        mn[:, j] = oh.transpose(1, 0, 2).astype(NP_BF16)   # [p, s, g]
        mnt[:, j] = oh.transpose(2, 0, 1).astype(NP_BF16)  # [g, s, p]
    return nfa, nft, mn, mnt


def kernel(node_feats, seg_ids, Wl, bl, Wp, bp, Wih, Whh, bih, bhh):
    node_feats = np.asarray(node_feats, np.float32)
    seg = np.asarray(seg_ids).astype(np.int64)
    Wl = np.asarray(Wl, np.float32)
    bl = np.asarray(bl, np.float32)
    Wp = np.asarray(Wp, np.float32)
    bp = np.asarray(bp, np.float32)
    Wih = np.asarray(Wih, np.float32)
    Whh = np.asarray(Whh, np.float32)
    bih = np.asarray(bih, np.float32)
    bhh = np.asarray(bhh, np.float32)
    V = node_feats.shape[0]

    bounds_g = [0]
    for c in range(1, NCORES):
        bounds_g.append(int(seg[c * V // NCORES]))
    bounds_g.append(G)

    NT_G = max((bounds_g[c + 1] - bounds_g[c] + 127) // 128 for c in range(NCORES))
    maxnodes = 1
    for c in range(NCORES):
        for gt in range(bounds_g[c], bounds_g[c + 1], 128):
            ge = min(gt + 128, bounds_g[c + 1])
            a = np.searchsorted(seg, gt, 'left')
            b = np.searchsorted(seg, ge, 'left')
            maxnodes = max(maxnodes, int(b - a))
    NSUB = (maxnodes + 127) // 128

    nc, ctx = _build_program(NT_G, NSUB, [float(bl[t, 0]) for t in range(T)])

    # replicated weight arrays
    fr = np.arange(128)
    wln2 = np.zeros((128, 2, T), np.float32)
    for t in range(T):
        for k in range(2):
            wln2[:, k, t] = Wl[t, 0, F + k * 128:F + (k + 1) * 128]
    wlg = np.zeros((128, T, F), np.float32)
    wpt = np.zeros((128, T, 2, F), np.float32)
    wih = np.zeros((128, T, 2, 3 * F), np.float32)
    whh = np.zeros((128, T, 2, 3 * F), np.float32)
    brz = np.zeros((1, T, 2 * F), np.float32)
    bin2 = np.zeros((1, T, F), np.float32)
    bhn = np.zeros((1, T, F), np.float32)
    bpr = np.zeros((1, T, F), np.float32)
    for t in range(T):
        wlg[:, t, :] = np.broadcast_to(Wl[t, 0, :F], (128, F))
        for k in range(2):
            wpt[:, t, k, :] = Wp[t][:, k * 128:(k + 1) * 128].T
            wih[:, t, k, :] = Wih[t][:, k * 128:(k + 1) * 128].T
            whh[:, t, k, :] = Whh[t][:, k * 128:(k + 1) * 128].T
        # n-gate input half pre-scaled x2 for the tanh(x/2) sigmoid trick
        wih[:, t, :, 2 * F:] *= 2.0
        # ctx is fed as ctx+1; subtract column sums of Wih from biases
        csum = Wih[t].sum(axis=1)  # [3F]
        brz[0, t, :] = bih[t, :2 * F] + bhh[t, :2 * F] - csum[:2 * F]
        bin2[0, t, :] = 2.0 * (bih[t, 2 * F:] - csum[2 * F:])
        bhn[0, t, :] = bhh[t, 2 * F:]
        bpr[0, t, :] = bp[t]
    shared = {
        "ones1": np.ones((1, 128), np.float32).astype(NP_BF16),
        "identb": np.eye(128, dtype=np.float32).astype(NP_BF16),
        "wln2": wln2.astype(NP_BF16), "wlg": wlg.astype(NP_BF16),
        "wpt": wpt.astype(NP_BF16), "wih": wih.astype(NP_BF16),
        "whh": whh.astype(NP_BF16), "brz": brz.astype(NP_BF16),
        "bin2": bin2.astype(NP_BF16), "bhn": bhn.astype(NP_BF16),
        "bp": bpr.astype(NP_BF16),
    }

    in_maps = []
    for c in range(NCORES):
        nfa, nft, mn, mnt = _prep_core(
            node_feats, seg, bounds_g[c], bounds_g[c + 1], NT_G, NSUB)
        m = dict(shared)
        m["nfa"] = nfa
        m["nft"] = nft
        m["mn"] = mn
        m["mnt"] = mnt
        in_maps.append(m)

    res = run_bass_kernel_spmd(nc, in_maps, core_ids=list(range(NCORES)))
    ctx.close()
    global LAST_RESULT
    LAST_RESULT = res

    out = np.zeros((G, F), np.float32)
    for c in range(NCORES):
        gc = bounds_g[c + 1] - bounds_g[c]
        out[bounds_g[c]:bounds_g[c + 1]] = res.results[c]["out"][:gc]
    return out
